# revision 35
# baseline (speedup 1.0000x reference)
"""CTC total-loss kernel for Trainium2 (8 NeuronCores, Bass/Tile).

Strategy (data-parallel over batch, 4 examples per core):

 * loss_b = -log(P_b) + tilt corrections + sum_{t<al} lse[t,b]; lse from
   per-(t,b) sum(exp(acts)); P_b from TWO unnormalized probability-domain
   lattice recursions that each cover HALF the lattice columns:
     - alpha: forward from s=0, columns s in [0, 32]
     - beta~: backward from the end states, stored re-indexed as
       sigma = 2L_b - s so its seeds sit at fixed columns (same layout
       as alpha); covers s in [2L-32, 2L].
   Host joins them in f64 over the s=32|33 boundary crossings:
     P = sum_t alpha_t[31]k[33]b~_{t+1}[33] + alpha_t[32](b~_{t+1}[33]
         + k[34]b~_{t+1}[34])   (+ alpha-side end term when 2L <= 32).
 * Both chains run as ONE wavefront on 32 partitions (alpha on 0..15,
   beta on 16..31; 4 examples x 4 time segments of 128 steps each).  The
   wave-aligned column storage makes both chains use identical column
   indices per wave, so each wave is THREE pure-DVE instructions:
     stream_shuffle  u[:,0]   <- boundary column, partition hop p->p+4
                               (chain-crossing wrap killed by E_0 = 0)
     scalar_tensor_tensor u[:,1:] = k*x[s-2] + x[s-1]
     tensor_tensor_scan   x[s] = (u + state)*E  (state seeds via slot 0)
   36 serial waves; DVE op cost is fixed-overhead dominated and scales
   with ceil(P/32), so the dual chain packs into one 32-partition block.
 * Emission table E comes from a Schraudolph exponential on the DVE
   (tensor_scalar g*s + (tilt*s + c) -> int16 at 4x rate, bitcast bf16),
   so the scalar engine never gates the chain.  Loader slots are biased
   to land exactly on 1.0; invalid cells land near int16 1000 -> 2^-119.
 * The lse stream is COMPACTED host-side to rows with t < act_len (~75%)
   and padded to whole tiles; the program is compiled for this input's
   worst-core tile count.  ACT tiles take int8 acts (quant step folded
   into the activation scale) with fused Exp+accum row sums; a few DVE
   tiles take bf16 acts via Schraudolph exp + two bf16 2x halving adds +
   a short accumulate pass.  int8/bf16 cut the HBM stream ~4x.
 * f32 dynamic range is controlled by per-(example, segment, direction)
   exponential tilts from normalized f64 proxy recursions host-side,
   folded back into the loss in log domain at finalize.
"""

import numpy as np

import concourse.bass as bass
import concourse.bacc as bacc
import concourse.tile as tile
from concourse import mybir

F32 = mybir.dt.float32
BF16 = mybir.dt.bfloat16
I8 = mybir.dt.int8
I16 = mybir.dt.int16

T, B, V, LMAX = 512, 32, 4096, 32
NCORES = 8
BC = B // NCORES            # 4 examples per core
S = 2 * LMAX + 1            # 65 lattice states
SHALF = 33                  # columns per half-lattice chain
H = 4                       # time segments
SEG = T // H                # 128 steps per segment
NW = SHALF + H - 1          # 36 anti-diagonal waves
CW = SEG + 1                # column width (slot 0 = boundary)
NCOL = NW + 2               # wave-aligned columns incl. 2 virtual leaders
P = 2 * BC * H              # 32 partitions: alpha 0..15, beta 16..31
ECH = 12                    # E chunk size in waves (3 chunks)

Q8 = 6.0 / 127.0            # int8 quantization step
S_BF = 184.6650292          # 128*log2(e)
C_TS = 16256.0 - 7.0        # Schraudolph bias (tuned on N(0,1) inputs)
G_ONE = (16256.0 - C_TS) / S_BF     # emission arg that lands exactly on 1.0
G_INV = (1000.0 - C_TS) / S_BF      # emission arg that lands on ~2^-119

_CACHE = {}


def _build_nc(ntu, yd):
    """Program for `ntu` stream tiles, the first `yd` on the DVE."""
    xa = ntu - yd
    nc = bacc.Bacc(None)
    acts8_d = nc.dram_tensor("acts8", [xa * 128, V], I8, kind="ExternalInput")
    actsb_d = nc.dram_tensor("actsb", [yd * 128, V], BF16,
                             kind="ExternalInput")
    gsub_d = nc.dram_tensor("gsub", [P, NW * CW], BF16, kind="ExternalInput")
    seedv_d = nc.dram_tensor("seedv", [P, 1], F32, kind="ExternalInput")
    skipk_d = nc.dram_tensor("skipk", [P, NW], F32, kind="ExternalInput")
    biasv_d = nc.dram_tensor("biasv", [P, 1], F32, kind="ExternalInput")
    xdump_d = nc.dram_tensor("xdump", [P, NCOL * CW], F32,
                             kind="ExternalOutput")
    sums_d = nc.dram_tensor("sums", [128, ntu], F32, kind="ExternalOutput")
    sums2_d = nc.dram_tensor("sums2", [128, 2], F32, kind="ExternalOutput")

    nch = (NW + ECH - 1) // ECH
    hop = [(i - BC) % 32 for i in range(32)]

    with tile.TileContext(nc) as tc:
        with (
            tc.tile_pool(name="small", bufs=1) as small,
            tc.tile_pool(name="big", bufs=1) as big,
            tc.tile_pool(name="gload", bufs=2) as gload,
            tc.tile_pool(name="astream", bufs=6) as astream,
            tc.tile_pool(name="dstream", bufs=2) as dstream,
            tc.tile_pool(name="i16p", bufs=2) as i16p,
            tc.tile_pool(name="sink", bufs=1) as sink,
        ):
            # ---------------- persistent tiles ----------------
            E = big.tile([P, NW * CW], I16)        # Schraudolph exp bits
            xall = big.tile([P, NCOL * CW], F32)   # wave-aligned columns
            u = big.tile([P, CW], F32)             # per-wave u term

            skipk_t = small.tile([P, NW], F32)
            nc.gpsimd.dma_start(out=skipk_t[:], in_=skipk_d[:])
            biasv_t = small.tile([P, 1], F32)
            nc.gpsimd.dma_start(out=biasv_t[:], in_=biasv_d[:])
            zbias = small.tile([128, 1], F32)
            nc.vector.memset(zbias[:], 0.0)
            sums = small.tile([128, ntu], F32)
            sums2 = small.tile([128, 2], F32)
            nc.vector.memset(sums[:, ntu - 1:ntu], 0.0)

            # init: zero the two virtual leader columns, then the
            # "alpha_{-1}" seeds (column 1, slot 0) for both chains --
            # via DMA, since engine ops can't start at partition 16.
            nc.vector.memset(xall[:, 0:2 * CW], 0.0)
            nc.gpsimd.dma_start(out=xall[:, CW:CW + 1], in_=seedv_d[:])

            # ------------- emissions in -> E (DVE Schraudolph) ----------
            def e_chunk(ci):
                w0 = ci * ECH
                w1 = min(NW, w0 + ECH)
                gch = gload.tile([P, ECH * CW], BF16, tag="gch")
                nc.sync.dma_start(out=gch[:, :(w1 - w0) * CW],
                                  in_=gsub_d[:, w0 * CW:w1 * CW])
                nc.vector.tensor_scalar(
                    out=E[:, w0 * CW:w1 * CW], in0=gch[:, :(w1 - w0) * CW],
                    scalar1=S_BF, scalar2=biasv_t[:],
                    op0=mybir.AluOpType.mult, op1=mybir.AluOpType.add)

            # ---------------- stream tiles by engine ----------------
            def a_tile(i, split=False):
                r0 = (i - yd) * 128
                xt = astream.tile([128, V], I8, tag="xa")
                ex = sink.tile([128, V], BF16, tag="exa")
                if not split:
                    nc.sync.dma_start(out=xt[:],
                                      in_=acts8_d[r0:r0 + 128, :])
                    nc.scalar.activation(
                        out=ex[:], in_=xt[:],
                        func=mybir.ActivationFunctionType.Exp,
                        bias=zbias[:], scale=Q8,
                        accum_out=sums[:, i:i + 1])
                else:
                    hv = V // 2
                    for q in range(2):
                        nc.sync.dma_start(
                            out=xt[:, q * hv:(q + 1) * hv],
                            in_=acts8_d[r0:r0 + 128, q * hv:(q + 1) * hv])
                    for q in range(2):
                        nc.scalar.activation(
                            out=ex[:, q * hv:(q + 1) * hv],
                            in_=xt[:, q * hv:(q + 1) * hv],
                            func=mybir.ActivationFunctionType.Exp,
                            bias=zbias[:], scale=Q8,
                            accum_out=sums2[:, q:q + 1])

            def d_dma(i):
                xt = dstream.tile([128, V], BF16, tag="xd")
                nc.sync.dma_start(out=xt[:],
                                  in_=actsb_d[i * 128:(i + 1) * 128, :])
                return xt

            def d_tile(i, xt):
                # Schraudolph exp at 4x DVE rate, two 2x halving adds,
                # then the (1x-rate) accumulate on a quarter tile.
                t16 = i16p.tile([128, V], I16, tag="td")
                nc.vector.tensor_scalar(
                    out=t16[:], in0=xt[:], scalar1=S_BF, scalar2=C_TS,
                    op0=mybir.AluOpType.mult, op1=mybir.AluOpType.add)
                eb = t16[:].bitcast(BF16)
                h1 = V // 2
                half = i16p.tile([128, h1], BF16, tag="th")
                nc.vector.tensor_tensor(
                    out=half[:], in0=eb[:, 0:h1], in1=eb[:, h1:V],
                    op=mybir.AluOpType.add)
                h2 = V // 4
                quart = i16p.tile([128, h2], BF16, tag="tq")
                nc.vector.tensor_tensor(
                    out=quart[:], in0=half[:, 0:h2], in1=half[:, h2:h1],
                    op=mybir.AluOpType.add)
                dmy = sink.tile([128, h2], BF16, tag="dmyd")
                nc.vector.tensor_scalar(
                    out=dmy[:], in0=quart[:],
                    scalar1=1.0, scalar2=None,
                    op0=mybir.AluOpType.mult, op1=mybir.AluOpType.add,
                    accum_out=sums[:, i:i + 1])

            # ---------------- issue order ----------------
            # DMA: two ACT tiles first (the scalar engine is the critical
            # path and must start ASAP), then the gsub chunks (chain
            # start), the rest of the int8 stream, DVE bf16 tiles last
            # (consumed only after the wave chain ends).
            act_idx = list(range(yd, ntu))
            a_tile(act_idx[0])
            e_chunk(0)
            a_tile(act_idx[1])
            e_chunk(1)
            e_chunk(2)
            for j in range(2, len(act_idx) - 1):
                a_tile(act_idx[j])
            a_tile(act_idx[-1], split=True)
            dve_tiles = [(i, d_dma(i)) for i in range(yd)]

            # ---------------- wavefront (pure DVE, both chains) --------
            for w in range(NW):
                nc.vector.stream_shuffle(
                    u[:, 0:1],
                    xall[:, (w + 1) * CW + SEG:(w + 1) * CW + SEG + 1],
                    hop)
                nc.vector.scalar_tensor_tensor(
                    out=u[:, 1:CW],
                    in0=xall[:, w * CW:w * CW + SEG],
                    scalar=skipk_t[:, w:w + 1],
                    in1=xall[:, (w + 1) * CW:(w + 1) * CW + SEG],
                    op0=mybir.AluOpType.mult,
                    op1=mybir.AluOpType.add)
                nc.vector.tensor_tensor_scan(
                    out=xall[:, (w + 2) * CW:(w + 3) * CW],
                    data0=u[:],
                    data1=E[:, w * CW:(w + 1) * CW].bitcast(BF16),
                    initial=0.0,
                    op0=mybir.AluOpType.add,
                    op1=mybir.AluOpType.mult)

            # xdump depends only on the chain -- issue before d_tiles.
            nc.sync.dma_start(out=xdump_d[:], in_=xall[:])

            for i, xt in dve_tiles:
                d_tile(i, xt)

            nc.sync.dma_start(out=sums_d[:], in_=sums[:])
            nc.sync.dma_start(out=sums2_d[:], in_=sums2[:])

    nc.compile()
    return nc


def _get_nc(ntu, yd):
    key = (ntu, yd)
    if key not in _CACHE:
        _CACHE[key] = _build_nc(ntu, yd)
    return _CACHE[key]


def _proxy_tilt(EG, Kf):
    """Normalized f64 recursion over (B, W) emission tables EG[t] -> per
    (example, segment) log-mass drift tilts (B, H)."""
    Bn = EG.shape[1]
    A = np.zeros((Bn, EG.shape[2]), np.float64)
    logm = np.zeros((Bn, T), np.float64)
    zer1 = np.zeros((Bn, 1), np.float64)
    zer2 = np.zeros((Bn, 2), np.float64)
    A[:, 0] = EG[0, :, 0]
    A[:, 1] = EG[0, :, 1]
    m = A.sum(1)
    m[m == 0] = 1.0
    A /= m[:, None]
    logm[:, 0] = np.log(m)
    for t in range(1, T):
        A1 = np.concatenate([zer1, A[:, :-1]], 1)
        A2 = np.concatenate([zer2, A[:, :-2]], 1)
        A = EG[t] * (A + A1 + Kf * A2)
        m = A.sum(1)
        m[m == 0] = 1.0
        A /= m[:, None]
        logm[:, t] = np.log(m)
    drift = logm.reshape(Bn, H, SEG).sum(2)
    return -drift / SEG


def host_prep(acts, labels, act_lens, label_lens):
    """Build the 8 per-core input maps + finalize aux data."""
    import ml_dtypes
    acts = np.ascontiguousarray(np.asarray(acts, dtype=np.float32))
    labels = np.asarray(labels).astype(np.int64)
    al = np.asarray(act_lens).astype(np.int64)
    ll = np.asarray(label_lens).astype(np.int64)
    offsets = np.cumsum(ll) - ll

    # lattice vocab ids EXT[b, s] and skip mask K[b, s]
    EXT = np.zeros((B, S), np.int64)
    K = np.zeros((B, S), np.float32)
    for b in range(B):
        L = int(ll[b])
        labp = np.zeros(LMAX, np.int64)
        labp[:L] = labels[offsets[b]:offsets[b] + L]
        EXT[b, 1::2] = labp
        K[b, 1] = 1.0
        for jj in range(1, L):
            if labp[jj] != labp[jj - 1]:
                K[b, 2 * jj + 1] = 1.0

    # G[t, b, s] = acts[t, b, EXT[b, s]]
    G = np.take_along_axis(acts, np.broadcast_to(EXT[None], (T, B, S)), axis=2)

    # ---- alpha tables: columns s in [0, 32] ----
    GA = G[:, :, :SHALF].astype(np.float64)              # (T, B, 33)
    EGA = np.exp(GA)
    for b in range(B):
        EGA[al[b]:, b, :] = 0.0
    KA = K[:, :SHALF].astype(np.float64)
    tilt_a = _proxy_tilt(EGA, KA)                        # (B, H)

    # ---- beta tables: sigma = 2L - s, time-reversed, act_len-aligned --
    GB = np.full((T, B, SHALF), -np.inf, np.float64)
    KB = np.zeros((B, SHALF), np.float64)
    for b in range(B):
        L = int(ll[b])
        a_b = int(al[b])
        sig = np.arange(SHALF)
        svals = 2 * L - sig
        ok2 = (svals + 2 <= 2 * L)
        KB[b, sig[ok2]] = K[b, svals[ok2] + 2]
        KB[b, 1] = 1.0
        tprime = a_b - np.arange(1, T + 1)
        vt = tprime >= 0
        GB[:a_b, b, :] = G[tprime[vt], b, :][:, svals]
    EGB = np.exp(GB)
    EGB[~np.isfinite(GB)] = 0.0
    tilt_b = _proxy_tilt(EGB, KB)                        # (B, H)

    # ---- balance examples across cores by act_len (greedy), then build
    # the compacted lse row plan (rows with t < act_len only) ----
    asg = [[] for _ in range(NCORES)]
    loads = np.zeros(NCORES, np.int64)
    for b in np.argsort(-al):
        elig = [k for k in range(NCORES) if len(asg[k]) < BC]
        k = min(elig, key=lambda q: loads[q])
        asg[k].append(int(b))
        loads[k] += al[b]
    core_rows = []
    for k in range(NCORES):
        tt, bb = [], []
        for t in range(T):
            for bl in range(BC):
                if t < al[asg[k][bl]]:
                    tt.append(t)
                    bb.append(bl)
        core_rows.append((np.array(tt), np.array(bb)))
    nmax = max(len(tt) for tt, _ in core_rows)
    ntu = (nmax + 127) // 128
    # engine balance from measured spans: ACT (starts ~8.7us, 3.97us per
    # full tile + 5.8 fixed) vs DVE (starts ~11us, chain 33.2us, 4.24us
    # per stream tile); last tile is the ACT split tile.
    q = int(round((36.4 + 4.24 * (ntu - 1) - 5.8) / 8.21))
    xa = min(max(q + 1, ntu - 6), ntu - 1)
    yd = ntu - xa

    in_maps = []
    for k in range(NCORES):
        tt, bb = core_rows[k]
        bmap = np.array(asg[k])
        rows = np.zeros((ntu * 128, V), np.float32)
        rows[:len(tt)] = acts[tt, bmap[bb], :]
        acts8 = np.clip(np.round(rows[yd * 128:] * (1.0 / Q8)), -127, 127) \
            .astype(np.int8)
        actsb = rows[:yd * 128].astype(ml_dtypes.bfloat16)

        # gsub holds the Schraudolph argument g such that
        # y = g*S_BF + (tilt*S_BF + C_TS); loaders hit exactly 1.0 and
        # invalid cells land near int16 1000 (~2^-119).
        gsub = np.zeros((P, NW, CW), np.float32)
        skipk = np.zeros((P, NW), np.float32)
        biasv = np.zeros((P, 1), np.float32)
        for h in range(H):
            for bl in range(BC):
                b = asg[k][bl]
                a_b = int(al[b])
                t0 = SEG * h
                nv = int(np.clip(a_b - t0, 0, SEG))
                for side, base, GT, tilt in (
                        (0, 0, GA, tilt_a), (1, 16, GB, tilt_b)):
                    p = base + BC * h + bl
                    tl = tilt[b, h]
                    biasv[p, 0] = tl * S_BF + C_TS
                    gsub[p, :, :] = G_INV - tl
                    if nv > 0:
                        if h > 0:
                            gsub[p, h:h + SHALF, 0] = G_ONE - tl
                        gt = GT[t0:t0 + nv, b, :]
                        gf = np.where(np.isfinite(gt), gt, G_INV - tl)
                        gsub[p, h:h + SHALF, 1:1 + nv] = gf.T
                    skipk[p, h:h + SHALF] = \
                        K[b, :SHALF] if side == 0 else KB[b, :]
        seedv = np.zeros((P, 1), np.float32)
        seedv[0:BC, 0] = 1.0
        seedv[16:16 + BC, 0] = 1.0
        in_maps.append({"acts8": acts8,
                        "actsb": actsb,
                        "gsub": gsub.reshape(P, NW * CW)
                                    .astype(ml_dtypes.bfloat16),
                        "skipk": skipk, "biasv": biasv, "seedv": seedv})
    aux = {"tilt_a": tilt_a, "tilt_b": tilt_b, "al": al, "ll": ll, "K": K,
           "core_rows": core_rows, "ntu": ntu, "yd": yd, "asg": asg}
    return in_maps, aux


def _chain_logs(xd, base_p, bl, cols, tilt_row):
    """log of the dumped chain values at the given wave-aligned lattice
    columns, un-tilted, as (T, len(cols)); invalid/<=0 -> -inf."""
    out = np.full((T, len(cols)), -np.inf)
    bsum = np.concatenate([[0.0], np.cumsum(np.repeat(tilt_row, SEG))])
    for j, scol in enumerate(cols):
        if scol < 0:
            continue
        for h in range(H):
            c = scol + h + 2
            part = base_p + BC * h + bl
            v = xd[part, c * CW + 1:c * CW + CW]
            pos = v > 0
            t0 = SEG * h
            out[t0:t0 + SEG, j][pos] = \
                np.log(v[pos]) - bsum[t0 + 1:t0 + SEG + 1][pos]
    return out


def example_loss(r, aux, k, bl):
    """Per-example loss from core k's outputs (f64). Returns (loss, dbg)."""
    tilt_a, tilt_b = aux["tilt_a"], aux["tilt_b"]
    al, ll, K = aux["al"], aux["ll"], aux["K"]
    ntu = aux["ntu"]
    tt, bb = aux["core_rows"][k]
    b = aux["asg"][k][bl]
    L = int(ll[b])
    a_b = int(al[b])
    xd = np.asarray(r["xdump"], np.float64)
    sums = np.asarray(r["sums"], np.float64)
    sums2 = np.asarray(r["sums2"], np.float64)
    flat = np.concatenate([sums.T.reshape(-1)[:(ntu - 1) * 128],
                           sums2[:, 0] + sums2[:, 1]])
    lse_full = np.zeros((T, BC))
    lse_full[tt, bb] = np.log(flat[:len(tt)])

    la = _chain_logs(xd, 0, bl, [31, 32], tilt_a[b])
    lbt = _chain_logs(xd, 16, bl, [2 * L - 33, 2 * L - 34], tilt_b[b])
    terms = []
    t = np.arange(0, a_b - 1)
    tau0 = a_b - 2 - t
    lb33 = lbt[tau0, 0]
    lb34 = lbt[tau0, 1]
    if 33 <= 2 * L and K[b, 33] > 0:
        terms.append(la[t, 0] + lb33)                      # 31 -skip-> 33
    if 33 <= 2 * L:
        terms.append(la[t, 1] + lb33)                      # 32 -step-> 33
    if 34 <= 2 * L and K[b, 34] > 0:
        terms.append(la[t, 1] + lb34)                      # 32 -skip-> 34
    if 2 * L <= 32:
        terms.append(np.array([la[a_b - 1, 0], la[a_b - 1, 1]]))
    allt = np.concatenate(terms) if terms else np.array([-np.inf])
    m = np.max(allt)
    if not np.isfinite(m):
        return None, m
    logp = m + np.log(np.sum(np.exp(allt - m)))
    return (-logp + lse_full[:a_b, bl].sum()), logp


def host_finalize(results, aux):
    """Assemble the scalar loss from per-core outputs."""
    total = np.float64(0.0)
    for k in range(NCORES):
        for bl in range(BC):
            loss_b, _ = example_loss(results[k], aux, k, bl)
            total += loss_b
    return np.array([total], dtype=np.float32)


def kernel(acts, labels, act_lens, label_lens):
    from concourse.bass_utils import run_bass_kernel_spmd
    in_maps, aux = host_prep(acts, labels, act_lens, label_lens)
    nc = _get_nc(aux["ntu"], aux["yd"])
    res = run_bass_kernel_spmd(nc, in_maps, list(range(NCORES)))
    return host_finalize(res.results, aux)


# revision 38
# speedup vs baseline: 1.0858x; 1.0858x over previous
"""CTC total-loss kernel for Trainium2 (8 NeuronCores, Bass/Tile).

Strategy (data-parallel over batch, 4 examples per core):

 * loss_b = -log(P_b) + tilt corrections + sum_{t<al} lse[t,b]; lse from
   per-(t,b) sum(exp(acts)); P_b from TWO unnormalized probability-domain
   lattice recursions that each cover HALF the lattice columns:
     - alpha: forward from s=0, columns s in [0, 32]
     - beta~: backward from the end states, stored re-indexed as
       sigma = 2L_b - s so its seeds sit at fixed columns (same layout
       as alpha); covers s in [2L-32, 2L].
   Host joins them in f64 over the s=32|33 boundary crossings:
     P = sum_t alpha_t[31]k[33]b~_{t+1}[33] + alpha_t[32](b~_{t+1}[33]
         + k[34]b~_{t+1}[34])   (+ alpha-side end term when 2L <= 32).
 * Both chains run as ONE wavefront on 32 partitions (alpha on 0..15,
   beta on 16..31; 4 examples x 4 time segments of 128 steps each).  The
   wave-aligned column storage makes both chains use identical column
   indices per wave, so each wave is THREE pure-DVE instructions:
     stream_shuffle  u[:,0]   <- boundary column, partition hop p->p+4
                               (chain-crossing wrap killed by E_0 = 0)
     scalar_tensor_tensor u[:,1:] = k*x[s-2] + x[s-1]
     tensor_tensor_scan   x[s] = (u + state)*E  (state seeds via slot 0)
   36 serial waves; DVE op cost is fixed-overhead dominated and scales
   with ceil(P/32), so the dual chain packs into one 32-partition block.
 * Emission table E comes from a Schraudolph exponential on the DVE
   (tensor_scalar g*s + (tilt*s + c) -> int16 at 4x rate, bitcast bf16),
   so the scalar engine never gates the chain.  Loader slots are biased
   to land exactly on 1.0; invalid cells land near int16 1000 -> 2^-119.
 * The lse stream is COMPACTED host-side to rows with t < act_len (~75%)
   and padded to whole tiles; the program is compiled for this input's
   worst-core tile count.  ACT tiles take int8 acts (quant step folded
   into the activation scale) with fused Exp+accum row sums; a few DVE
   tiles take bf16 acts via Schraudolph exp + two bf16 2x halving adds +
   a short accumulate pass.  int8/bf16 cut the HBM stream ~4x.
 * f32 dynamic range is controlled by per-(example, segment, direction)
   exponential tilts from normalized f64 proxy recursions host-side,
   folded back into the loss in log domain at finalize.
"""

import numpy as np

import concourse.bass as bass
import concourse.bacc as bacc
import concourse.tile as tile
from concourse import mybir

F32 = mybir.dt.float32
BF16 = mybir.dt.bfloat16
I8 = mybir.dt.int8
I16 = mybir.dt.int16

T, B, V, LMAX = 512, 32, 4096, 32
NCORES = 8
BC = B // NCORES            # 4 examples per core
S = 2 * LMAX + 1            # 65 lattice states
SHALF = 33                  # columns per half-lattice chain
H = 4                       # time segments
SEG = T // H                # 128 steps per segment
NW = SHALF + H - 1          # 36 anti-diagonal waves
CW = SEG + 1                # column width (slot 0 = boundary)
NCOL = NW + 2               # wave-aligned columns incl. 2 virtual leaders
P = 2 * BC * H              # 32 partitions: alpha 0..15, beta 16..31
ECH = 12                    # E chunk size in waves (3 chunks)

Q8 = 6.0 / 127.0            # int8 quantization step
S_BF = 184.6650292          # 128*log2(e)
C_TS = 16256.0 - 7.0        # Schraudolph bias (tuned on N(0,1) inputs)
G_ONE = (16256.0 - C_TS) / S_BF     # emission arg that lands exactly on 1.0
G_INV = (1000.0 - C_TS) / S_BF      # emission arg that lands on ~2^-119

_CACHE = {}


def _build_nc(ntu, yd):
    """Program for `ntu` stream tiles, the first `yd` on the DVE."""
    xa = ntu - yd
    nc = bacc.Bacc(None)
    acts8_d = nc.dram_tensor("acts8", [xa * 128, V], I8, kind="ExternalInput")
    actsb_d = nc.dram_tensor("actsb", [yd * 128, V], BF16,
                             kind="ExternalInput")
    gsub_d = nc.dram_tensor("gsub", [P, NW * CW], BF16, kind="ExternalInput")
    seedv_d = nc.dram_tensor("seedv", [P, 1], F32, kind="ExternalInput")
    skipk_d = nc.dram_tensor("skipk", [P, NW], F32, kind="ExternalInput")
    biasv_d = nc.dram_tensor("biasv", [P, 1], F32, kind="ExternalInput")
    xdump_d = nc.dram_tensor("xdump", [P, NCOL * CW], F32,
                             kind="ExternalOutput")
    sums_d = nc.dram_tensor("sums", [128, ntu], F32, kind="ExternalOutput")
    sums2_d = nc.dram_tensor("sums2", [128, 2], F32, kind="ExternalOutput")

    nch = (NW + ECH - 1) // ECH
    hop = [(i - BC) % 32 for i in range(32)]

    with tile.TileContext(nc) as tc:
        with (
            tc.tile_pool(name="small", bufs=1) as small,
            tc.tile_pool(name="big", bufs=1) as big,
            tc.tile_pool(name="gload", bufs=2) as gload,
            tc.tile_pool(name="astream", bufs=6) as astream,
            tc.tile_pool(name="dstream", bufs=2) as dstream,
            tc.tile_pool(name="i16p", bufs=2) as i16p,
            tc.tile_pool(name="sink", bufs=1) as sink,
        ):
            # ---------------- persistent tiles ----------------
            E = big.tile([P, NW * CW], I16)        # Schraudolph exp bits
            xall = big.tile([P, NCOL * CW], F32)   # wave-aligned columns
            u = big.tile([P, CW], F32)             # per-wave u term

            skipk_t = small.tile([P, NW], F32)
            nc.gpsimd.dma_start(out=skipk_t[:], in_=skipk_d[:])
            biasv_t = small.tile([P, 1], F32)
            nc.gpsimd.dma_start(out=biasv_t[:], in_=biasv_d[:])
            zbias = small.tile([128, 1], F32)
            nc.vector.memset(zbias[:], 0.0)
            sums = small.tile([128, ntu], F32)
            sums2 = small.tile([128, 2], F32)
            # the split tile uses sums2; its sums column is only dumped
            nc.vector.memset(sums[:, yd:yd + 1], 0.0)
            # prewarm the Exp table so the first stream exp needs no
            # ACT_TABLE_LOAD on its critical path
            pwarm = small.tile([128, 1], BF16)
            nc.scalar.activation(
                out=pwarm[:], in_=zbias[:],
                func=mybir.ActivationFunctionType.Exp,
                bias=zbias[:], scale=1.0)

            # init: zero the two virtual leader columns, then the
            # "alpha_{-1}" seeds (column 1, slot 0) for both chains --
            # via DMA, since engine ops can't start at partition 16.
            nc.vector.memset(xall[:, 0:2 * CW], 0.0)
            nc.gpsimd.dma_start(out=xall[:, CW:CW + 1], in_=seedv_d[:])

            # ------------- emissions in -> E (DVE Schraudolph) ----------
            def e_chunk(ci):
                w0 = ci * ECH
                w1 = min(NW, w0 + ECH)
                gch = gload.tile([P, ECH * CW], BF16, tag="gch")
                nc.sync.dma_start(out=gch[:, :(w1 - w0) * CW],
                                  in_=gsub_d[:, w0 * CW:w1 * CW])
                nc.vector.tensor_scalar(
                    out=E[:, w0 * CW:w1 * CW], in0=gch[:, :(w1 - w0) * CW],
                    scalar1=S_BF, scalar2=biasv_t[:],
                    op0=mybir.AluOpType.mult, op1=mybir.AluOpType.add)

            # ---------------- stream tiles by engine ----------------
            def a_tile(i, split=False):
                r0 = (i - yd) * 128
                xt = astream.tile([128, V], I8, tag="xa")
                ex = sink.tile([128, V], BF16, tag="exa")
                if not split:
                    nc.sync.dma_start(out=xt[:],
                                      in_=acts8_d[r0:r0 + 128, :])
                    nc.scalar.activation(
                        out=ex[:], in_=xt[:],
                        func=mybir.ActivationFunctionType.Exp,
                        bias=zbias[:], scale=Q8,
                        accum_out=sums[:, i:i + 1])
                else:
                    hv = V // 2
                    for q in range(2):
                        nc.sync.dma_start(
                            out=xt[:, q * hv:(q + 1) * hv],
                            in_=acts8_d[r0:r0 + 128, q * hv:(q + 1) * hv])
                    for q in range(2):
                        nc.scalar.activation(
                            out=ex[:, q * hv:(q + 1) * hv],
                            in_=xt[:, q * hv:(q + 1) * hv],
                            func=mybir.ActivationFunctionType.Exp,
                            bias=zbias[:], scale=Q8,
                            accum_out=sums2[:, q:q + 1])

            def d_dma(i):
                xt = dstream.tile([128, V], BF16, tag="xd")
                nc.sync.dma_start(out=xt[:],
                                  in_=actsb_d[i * 128:(i + 1) * 128, :])
                return xt

            def d_tile(i, xt):
                # Schraudolph exp at 4x DVE rate, two 2x halving adds,
                # then the (1x-rate) accumulate on a quarter tile.
                t16 = i16p.tile([128, V], I16, tag="td")
                nc.vector.tensor_scalar(
                    out=t16[:], in0=xt[:], scalar1=S_BF, scalar2=C_TS,
                    op0=mybir.AluOpType.mult, op1=mybir.AluOpType.add)
                eb = t16[:].bitcast(BF16)
                h1 = V // 2
                half = i16p.tile([128, h1], BF16, tag="th")
                nc.vector.tensor_tensor(
                    out=half[:], in0=eb[:, 0:h1], in1=eb[:, h1:V],
                    op=mybir.AluOpType.add)
                h2 = V // 4
                quart = i16p.tile([128, h2], BF16, tag="tq")
                nc.vector.tensor_tensor(
                    out=quart[:], in0=half[:, 0:h2], in1=half[:, h2:h1],
                    op=mybir.AluOpType.add)
                dmy = sink.tile([128, h2], BF16, tag="dmyd")
                nc.vector.tensor_scalar(
                    out=dmy[:], in0=quart[:],
                    scalar1=1.0, scalar2=None,
                    op0=mybir.AluOpType.mult, op1=mybir.AluOpType.add,
                    accum_out=sums[:, i:i + 1])

            # ---------------- issue order ----------------
            # DMA: two ACT tiles first (the scalar engine is the critical
            # path and must start ASAP), then the gsub chunks (chain
            # start), the rest of the int8 stream, DVE bf16 tiles last
            # (consumed only after the wave chain ends).
            act_idx = list(range(yd, ntu))
            a_tile(act_idx[0], split=True)
            e_chunk(0)
            a_tile(act_idx[1])
            e_chunk(1)
            e_chunk(2)
            for j in range(2, len(act_idx)):
                a_tile(act_idx[j])
            dve_tiles = [(i, d_dma(i)) for i in range(yd)]

            # ---------------- wavefront (pure DVE, both chains) --------
            for w in range(NW):
                nc.vector.stream_shuffle(
                    u[:, 0:1],
                    xall[:, (w + 1) * CW + SEG:(w + 1) * CW + SEG + 1],
                    hop)
                nc.vector.scalar_tensor_tensor(
                    out=u[:, 1:CW],
                    in0=xall[:, w * CW:w * CW + SEG],
                    scalar=skipk_t[:, w:w + 1],
                    in1=xall[:, (w + 1) * CW:(w + 1) * CW + SEG],
                    op0=mybir.AluOpType.mult,
                    op1=mybir.AluOpType.add)
                nc.vector.tensor_tensor_scan(
                    out=xall[:, (w + 2) * CW:(w + 3) * CW],
                    data0=u[:],
                    data1=E[:, w * CW:(w + 1) * CW].bitcast(BF16),
                    initial=0.0,
                    op0=mybir.AluOpType.add,
                    op1=mybir.AluOpType.mult)

            # xdump depends only on the chain -- issue before d_tiles.
            nc.sync.dma_start(out=xdump_d[:], in_=xall[:])

            for i, xt in dve_tiles:
                d_tile(i, xt)

            nc.sync.dma_start(out=sums_d[:], in_=sums[:])
            nc.sync.dma_start(out=sums2_d[:], in_=sums2[:])

    nc.compile()
    return nc


def _get_nc(ntu, yd):
    key = (ntu, yd)
    if key not in _CACHE:
        _CACHE[key] = _build_nc(ntu, yd)
    return _CACHE[key]


def _proxy_tilt(EG, Kf):
    """Normalized f64 recursion over (B, W) emission tables EG[t] -> per
    (example, segment) log-mass drift tilts (B, H)."""
    Bn = EG.shape[1]
    A = np.zeros((Bn, EG.shape[2]), np.float64)
    logm = np.zeros((Bn, T), np.float64)
    zer1 = np.zeros((Bn, 1), np.float64)
    zer2 = np.zeros((Bn, 2), np.float64)
    A[:, 0] = EG[0, :, 0]
    A[:, 1] = EG[0, :, 1]
    m = A.sum(1)
    m[m == 0] = 1.0
    A /= m[:, None]
    logm[:, 0] = np.log(m)
    for t in range(1, T):
        A1 = np.concatenate([zer1, A[:, :-1]], 1)
        A2 = np.concatenate([zer2, A[:, :-2]], 1)
        A = EG[t] * (A + A1 + Kf * A2)
        m = A.sum(1)
        m[m == 0] = 1.0
        A /= m[:, None]
        logm[:, t] = np.log(m)
    drift = logm.reshape(Bn, H, SEG).sum(2)
    return -drift / SEG


def host_prep(acts, labels, act_lens, label_lens):
    """Build the 8 per-core input maps + finalize aux data."""
    import ml_dtypes
    acts = np.ascontiguousarray(np.asarray(acts, dtype=np.float32))
    labels = np.asarray(labels).astype(np.int64)
    al = np.asarray(act_lens).astype(np.int64)
    ll = np.asarray(label_lens).astype(np.int64)
    offsets = np.cumsum(ll) - ll

    # lattice vocab ids EXT[b, s] and skip mask K[b, s]
    EXT = np.zeros((B, S), np.int64)
    K = np.zeros((B, S), np.float32)
    for b in range(B):
        L = int(ll[b])
        labp = np.zeros(LMAX, np.int64)
        labp[:L] = labels[offsets[b]:offsets[b] + L]
        EXT[b, 1::2] = labp
        K[b, 1] = 1.0
        for jj in range(1, L):
            if labp[jj] != labp[jj - 1]:
                K[b, 2 * jj + 1] = 1.0

    # G[t, b, s] = acts[t, b, EXT[b, s]]
    G = np.take_along_axis(acts, np.broadcast_to(EXT[None], (T, B, S)), axis=2)

    # ---- alpha tables: columns s in [0, 32] ----
    GA = G[:, :, :SHALF].astype(np.float64)              # (T, B, 33)
    EGA = np.exp(GA)
    for b in range(B):
        EGA[al[b]:, b, :] = 0.0
    KA = K[:, :SHALF].astype(np.float64)
    tilt_a = _proxy_tilt(EGA, KA)                        # (B, H)

    # ---- beta tables: sigma = 2L - s, time-reversed, act_len-aligned --
    GB = np.full((T, B, SHALF), -np.inf, np.float64)
    KB = np.zeros((B, SHALF), np.float64)
    for b in range(B):
        L = int(ll[b])
        a_b = int(al[b])
        sig = np.arange(SHALF)
        svals = 2 * L - sig
        ok2 = (svals + 2 <= 2 * L)
        KB[b, sig[ok2]] = K[b, svals[ok2] + 2]
        KB[b, 1] = 1.0
        tprime = a_b - np.arange(1, T + 1)
        vt = tprime >= 0
        GB[:a_b, b, :] = G[tprime[vt], b, :][:, svals]
    EGB = np.exp(GB)
    EGB[~np.isfinite(GB)] = 0.0
    tilt_b = _proxy_tilt(EGB, KB)                        # (B, H)

    # ---- balance examples across cores by act_len (greedy), then build
    # the compacted lse row plan (rows with t < act_len only) ----
    asg = [[] for _ in range(NCORES)]
    loads = np.zeros(NCORES, np.int64)
    for b in np.argsort(-al):
        elig = [k for k in range(NCORES) if len(asg[k]) < BC]
        k = min(elig, key=lambda q: loads[q])
        asg[k].append(int(b))
        loads[k] += al[b]
    core_rows = []
    for k in range(NCORES):
        tt, bb = [], []
        for t in range(T):
            for bl in range(BC):
                if t < al[asg[k][bl]]:
                    tt.append(t)
                    bb.append(bl)
        core_rows.append((np.array(tt), np.array(bb)))
    nmax = max(len(tt) for tt, _ in core_rows)
    ntu = (nmax + 127) // 128
    # engine balance from measured spans: ACT (starts ~8.7us, 3.97us per
    # full tile + 5.8 fixed) vs DVE (starts ~11us, chain 33.2us, 4.24us
    # per stream tile); last tile is the ACT split tile.
    q = int(round((36.4 + 4.24 * (ntu - 1) - 5.8) / 8.21))
    xa = min(max(q + 1, ntu - 6), ntu - 1)
    yd = ntu - xa

    in_maps = []
    for k in range(NCORES):
        tt, bb = core_rows[k]
        bmap = np.array(asg[k])
        rows = np.zeros((ntu * 128, V), np.float32)
        rows[:len(tt)] = acts[tt, bmap[bb], :]
        acts8 = np.clip(np.round(rows[yd * 128:] * (1.0 / Q8)), -127, 127) \
            .astype(np.int8)
        actsb = rows[:yd * 128].astype(ml_dtypes.bfloat16)

        # gsub holds the Schraudolph argument g such that
        # y = g*S_BF + (tilt*S_BF + C_TS); loaders hit exactly 1.0 and
        # invalid cells land near int16 1000 (~2^-119).
        gsub = np.zeros((P, NW, CW), np.float32)
        skipk = np.zeros((P, NW), np.float32)
        biasv = np.zeros((P, 1), np.float32)
        for h in range(H):
            for bl in range(BC):
                b = asg[k][bl]
                a_b = int(al[b])
                t0 = SEG * h
                nv = int(np.clip(a_b - t0, 0, SEG))
                for side, base, GT, tilt in (
                        (0, 0, GA, tilt_a), (1, 16, GB, tilt_b)):
                    p = base + BC * h + bl
                    tl = tilt[b, h]
                    biasv[p, 0] = tl * S_BF + C_TS
                    gsub[p, :, :] = G_INV - tl
                    if nv > 0:
                        if h > 0:
                            gsub[p, h:h + SHALF, 0] = G_ONE - tl
                        gt = GT[t0:t0 + nv, b, :]
                        gf = np.where(np.isfinite(gt), gt, G_INV - tl)
                        gsub[p, h:h + SHALF, 1:1 + nv] = gf.T
                    skipk[p, h:h + SHALF] = \
                        K[b, :SHALF] if side == 0 else KB[b, :]
        seedv = np.zeros((P, 1), np.float32)
        seedv[0:BC, 0] = 1.0
        seedv[16:16 + BC, 0] = 1.0
        in_maps.append({"acts8": acts8,
                        "actsb": actsb,
                        "gsub": gsub.reshape(P, NW * CW)
                                    .astype(ml_dtypes.bfloat16),
                        "skipk": skipk, "biasv": biasv, "seedv": seedv})
    aux = {"tilt_a": tilt_a, "tilt_b": tilt_b, "al": al, "ll": ll, "K": K,
           "core_rows": core_rows, "ntu": ntu, "yd": yd, "asg": asg}
    return in_maps, aux


def _chain_logs(xd, base_p, bl, cols, tilt_row):
    """log of the dumped chain values at the given wave-aligned lattice
    columns, un-tilted, as (T, len(cols)); invalid/<=0 -> -inf."""
    out = np.full((T, len(cols)), -np.inf)
    bsum = np.concatenate([[0.0], np.cumsum(np.repeat(tilt_row, SEG))])
    for j, scol in enumerate(cols):
        if scol < 0:
            continue
        for h in range(H):
            c = scol + h + 2
            part = base_p + BC * h + bl
            v = xd[part, c * CW + 1:c * CW + CW]
            pos = v > 0
            t0 = SEG * h
            out[t0:t0 + SEG, j][pos] = \
                np.log(v[pos]) - bsum[t0 + 1:t0 + SEG + 1][pos]
    return out


def example_loss(r, aux, k, bl):
    """Per-example loss from core k's outputs (f64). Returns (loss, dbg)."""
    tilt_a, tilt_b = aux["tilt_a"], aux["tilt_b"]
    al, ll, K = aux["al"], aux["ll"], aux["K"]
    ntu = aux["ntu"]
    tt, bb = aux["core_rows"][k]
    b = aux["asg"][k][bl]
    L = int(ll[b])
    a_b = int(al[b])
    xd = np.asarray(r["xdump"], np.float64)
    sums = np.asarray(r["sums"], np.float64)
    sums2 = np.asarray(r["sums2"], np.float64)
    yd = aux["yd"]
    flat = sums.T.copy().reshape(-1)
    flat[yd * 128:(yd + 1) * 128] = sums2[:, 0] + sums2[:, 1]
    lse_full = np.zeros((T, BC))
    lse_full[tt, bb] = np.log(flat[:len(tt)])

    la = _chain_logs(xd, 0, bl, [31, 32], tilt_a[b])
    lbt = _chain_logs(xd, 16, bl, [2 * L - 33, 2 * L - 34], tilt_b[b])
    terms = []
    t = np.arange(0, a_b - 1)
    tau0 = a_b - 2 - t
    lb33 = lbt[tau0, 0]
    lb34 = lbt[tau0, 1]
    if 33 <= 2 * L and K[b, 33] > 0:
        terms.append(la[t, 0] + lb33)                      # 31 -skip-> 33
    if 33 <= 2 * L:
        terms.append(la[t, 1] + lb33)                      # 32 -step-> 33
    if 34 <= 2 * L and K[b, 34] > 0:
        terms.append(la[t, 1] + lb34)                      # 32 -skip-> 34
    if 2 * L <= 32:
        terms.append(np.array([la[a_b - 1, 0], la[a_b - 1, 1]]))
    allt = np.concatenate(terms) if terms else np.array([-np.inf])
    m = np.max(allt)
    if not np.isfinite(m):
        return None, m
    logp = m + np.log(np.sum(np.exp(allt - m)))
    return (-logp + lse_full[:a_b, bl].sum()), logp


def host_finalize(results, aux):
    """Assemble the scalar loss from per-core outputs."""
    total = np.float64(0.0)
    for k in range(NCORES):
        for bl in range(BC):
            loss_b, _ = example_loss(results[k], aux, k, bl)
            total += loss_b
    return np.array([total], dtype=np.float32)


def kernel(acts, labels, act_lens, label_lens):
    from concourse.bass_utils import run_bass_kernel_spmd
    in_maps, aux = host_prep(acts, labels, act_lens, label_lens)
    nc = _get_nc(aux["ntu"], aux["yd"])
    res = run_bass_kernel_spmd(nc, in_maps, list(range(NCORES)))
    return host_finalize(res.results, aux)
